# revision 11
# baseline (speedup 1.0000x reference)
"""Trainium2 Bass kernel for GQA attention with RoPE (dense transformer).

Problem: B=2, S=2048, H=2048, 16 query heads / 4 KV heads, head_dim 128,
causal flash-style attention, fused QKV + o_proj.

Sharding (8 cores): (batch, head-group) grid. Core c handles batch c//4 and
head group c%4 (4 query heads + their shared KV head). o_proj is computed as
per-group partials reduced on host (tensor-parallel o_proj input split).

v2 layout/schedule (vs v1 336us baseline):
  - All SBUF operands and DMA traffic in bf16 (PSUM accumulation stays f32);
    halves HBM traffic and avoids the fp32r small-matmul (free dim < 256)
    4x PE penalty on diagonal score tiles.
  - Softmax row-sums no longer burn PE columns per k-tile: exp tiles are
    accumulated across k-tiles on the Vector engine (acc += ex), then ONE
    ones-matmul per (head, q-chunk) reduces partitions.
  - V is transposed to natural [tok, d] layout with the DMA xbar (16-bit
    transpose) instead of PE transposes.
  - o_proj for q-chunk qc-1 is interleaved into attention(qc)'s k-tile loop:
    the attention stretch is paced by Scalar EXP (~650ns/tile vs ~430ns of
    PE work), so the o_proj matmuls fill the PE bubbles and output DMAs
    stream out through the whole kernel instead of a tail phase.
  - Attention j-loop processes all 4 heads per k-tile with AV accumulating
    into 4 pinned PSUM banks, so each exp tile dies right after its AV
    matmul (ex pool stays at 3 bufs).
"""
import math

import ml_dtypes
import numpy as np

import concourse.bass as bass
import concourse.mybir as mybir
import concourse.tile as tile
from concourse import bacc
from concourse.bass_utils import run_bass_kernel_spmd

B, S, H = 2, 2048, 2048
NH, KVH, HD = 16, 4, 128
G = 4                 # head groups (= KVH); grid = G x B = 8 cores
GQ = NH // KVH        # query heads per group
QD = GQ * HD          # per-core q dim (512)
KC = H // 128         # contraction chunks for projections (16)
TC = 4                # token chunks of 512
TT = S // 128         # 128-token tiles (16)

F32 = mybir.dt.float32
F32R = mybir.dt.float32r
BF16 = mybir.dt.bfloat16
AF = mybir.ActivationFunctionType

_NC = None


def _emit(nc):
    # weights come host-packed with the SBUF partition dim outermost so each
    # load is one DMA with 128 large contiguous descriptors
    xT = nc.dram_tensor("xT", [H, S], BF16, kind="ExternalInput").ap()
    wqD = nc.dram_tensor("wqD", [128, KC * QD], BF16,
                         kind="ExternalInput").ap()
    wkvD = nc.dram_tensor("wkvD", [128, KC * 2 * HD], BF16,
                          kind="ExternalInput").ap()
    woD = nc.dram_tensor("woD", [128, GQ * H], BF16,
                         kind="ExternalInput").ap()
    cosT = nc.dram_tensor("cosT", [HD, S], F32, kind="ExternalInput").ap()
    sinS = nc.dram_tensor("sinS", [HD, S], F32, kind="ExternalInput").ap()
    bqkv = nc.dram_tensor("bqkv", [128, 6], F32, kind="ExternalInput").ap()
    onesd = nc.dram_tensor("onesd", [128, 128], BF16, kind="ExternalInput").ap()
    outp = nc.dram_tensor("outp", [S, H], BF16, kind="ExternalOutput").ap()

    xT3 = xT.rearrange("(ko p) t -> p ko t", p=128)
    wqD3 = wqD.rearrange("p (ko m) -> p ko m", ko=KC)
    wkvD3 = wkvD.rearrange("p (ko m) -> p ko m", ko=KC)
    woD3 = woD.rearrange("p (ic o) -> p ic o", ic=GQ)

    with tile.TileContext(nc) as tc:
        with (
            tc.tile_pool(name="persist", bufs=1) as pp,
            tc.tile_pool(name="qfp", bufs=2) as pqf,
            tc.tile_pool(name="accp", bufs=1) as pacc,
            tc.tile_pool(name="expp", bufs=1) as pex,
            tc.tile_pool(name="projx", bufs=1) as px,
            tc.tile_pool(name="rope", bufs=1) as pr,
            tc.tile_pool(name="outp", bufs=1) as pfo,
            tc.tile_pool(name="psum8", bufs=1, space="PSUM") as ps8,
        ):
            # persistent per-chunk K/V (split per t-chunk to keep dep ranges
            # disjoint between the producing chunk and attention readers)
            kf = [pp.tile([128, 512], BF16, name=f"kf{t}") for t in range(TC)]
            v_sb = [pp.tile([128, 4, HD], BF16, name=f"vsb{t}")
                    for t in range(TC)]
            ofl = pp.tile([128, GQ, S], BF16, name="ofl")

            # ---- constants / weights (all loads on the gpsimd swdge queue
            # so the sync queue carries only x chunks + outputs) ----
            bias_sb = pp.tile([128, 6], F32, name="bias")
            nc.gpsimd.dma_start(bias_sb[:, :], bqkv)
            ones_mat = pp.tile([128, 128], BF16, name="ones")
            nc.gpsimd.dma_start(ones_mat[:, :], onesd)
            wq_sb = pp.tile([128, KC, QD], BF16, name="wq")
            wkv_sb = pp.tile([128, KC, 2 * HD], BF16, name="wkv")
            wo_sb = pp.tile([128, GQ, H], BF16, name="wo")
            cos_sb = pp.tile([128, S], F32, name="cos")
            sin_sb = pp.tile([128, S], F32, name="sin")

            def oproj_blocks(qc):
                """16 emit-callables, one [128tok x 512out] PSUM block each."""
                blocks = []
                for tt in range(4 * qc, 4 * qc + 4):
                    for oc in range(4):
                        def blk(tt=tt, oc=oc, tag="OP"):
                            tsl = slice(128 * tt, 128 * tt + 128)
                            osl = slice(512 * oc, 512 * oc + 512)
                            pf = ps8.tile([128, 512], F32, tag=tag,
                                          bufs=(1 if tag == "OP" else 3),
                                          name=f"pf_{tt}_{oc}")
                            for ic in range(GQ):
                                nc.tensor.matmul(
                                    pf[:, :], ofl[:, ic, tsl],
                                    wo_sb[:, ic, osl],
                                    start=(ic == 0), stop=(ic == GQ - 1))
                            fo = pfo.tile([128, 512], BF16, tag="fo", bufs=4,
                                          name=f"fo_{tt}_{oc}")
                            nc.scalar.copy(fo[:, :], pf[:, :])
                            # alternate output queues so the drain isn't
                            # single-queue bound at the tail
                            eng = nc.sync if (tt + oc) % 2 == 0 else nc.gpsimd
                            eng.dma_start(outp[tsl, osl], fo[:, :])
                        blocks.append(blk)
                return blocks

            def attention(qc, qf_t, blocks):
                """flash attention for q-chunk qc over k-tiles 0..4qc+3,
                with o_proj blocks (prev chunk) interleaved as PE filler."""
                qs = slice(512 * qc, 512 * qc + 512)
                nj = 4 * qc + 4
                acc = pacc.tile([128, GQ, 512], BF16, tag="acc", bufs=2,
                                name=f"acc_{qc}")
                po = [ps8.tile([128, 512], F32, tag=f"PO{h}", bufs=1,
                               name=f"po_{qc}_{h}") for h in range(GQ)]
                emitted = 0
                for j in range(nj):
                    off = 0 if j < 4 * qc else 128 * j - 512 * qc
                    n = 512 - off
                    ex = pex.tile([128, GQ, 512], BF16, tag="E", bufs=3,
                                  name=f"ex_{qc}_{j}")
                    for h in range(GQ):
                        ps = ps8.tile([128, 512], F32, tag="A", bufs=3,
                                      name=f"ps_{qc}_{j}_{h}")
                        nc.tensor.matmul(
                            ps[:, 0:n],
                            kf[j // 4][:, 128 * (j % 4):128 * (j % 4) + 128],
                            qf_t[:, h, off:off + n], start=True, stop=True)
                        nc.scalar.activation(ex[:, h, 0:n], ps[:, 0:n],
                                             AF.Exp)
                    if j >= 4 * qc:
                        # zero the strictly-lower (q < k) triangle, all heads
                        nc.gpsimd.affine_select(
                            out=ex[:, :, 0:128], in_=ex[:, :, 0:128],
                            compare_op=mybir.AluOpType.is_ge, fill=0.0,
                            base=0, pattern=[[0, GQ], [1, 128]],
                            channel_multiplier=-1)
                    if j == 0:
                        nc.vector.tensor_copy(acc[:, :, :], ex[:, :, :])
                    else:
                        nc.vector.tensor_add(acc[:, :, off:512],
                                             acc[:, :, off:512],
                                             ex[:, :, 0:n])
                    for h in range(GQ):
                        nc.tensor.matmul(
                            po[h][:, off:off + n],
                            v_sb[j // 4][:, j % 4, :],
                            ex[:, h, 0:n],
                            start=(j == 0), stop=(j == nj - 1))
                    # interleave o_proj blocks of the previous q-chunk
                    want = (j + 1) * len(blocks) // nj
                    while emitted < want:
                        blocks[emitted]()
                        emitted += 1
                for h in range(GQ):
                    psum = ps8.tile([128, 512], F32, tag="A", bufs=3,
                                    name=f"psum_{qc}_{h}")
                    nc.tensor.matmul(psum[:, :], ones_mat[:, :],
                                     acc[:, h, :], start=True, stop=True)
                    bc = pr.tile([128, 512], F32, tag="bc", bufs=2,
                                 name=f"bc_{qc}_{h}")
                    nc.vector.reciprocal_approx_fast(bc[:, :], psum[:, :])
                    nc.vector.tensor_mul(ofl[:, h, qs], po[h][:, :],
                                         bc[:, :])

            def proj_chunk(t):
                """qkv projections + RoPE for token chunk t; returns qf."""
                ts = slice(512 * t, 512 * t + 512)
                xcs = []
                for ko in range(KC):
                    xc = px.tile([128, 512], BF16, tag="xc", bufs=36,
                                 name=f"xc_{t}_{ko}")
                    nc.sync.dma_start(xc[:, :], xT3[:, ko, ts])
                    if t == 0 and ko == 0:
                        nc.gpsimd.dma_start(wkv_sb[:, :, :], wkvD3)
                        nc.gpsimd.dma_start(wq_sb[:, :, :], wqD3)
                        nc.gpsimd.dma_start(cos_sb[:, :], cosT)
                        nc.gpsimd.dma_start(sin_sb[:, :], sinS)
                    if t == 1 and ko == 0:
                        nc.gpsimd.dma_start(wo_sb[:, :, :], woD3)
                    xcs.append(xc)

                qf_t = pqf.tile([128, GQ, 512], BF16, tag="qf",
                                name=f"qf_{t}")
                # block-serial projections: one PSUM bank at a time so the
                # A-ring stays shallow; k+v first (their weights land first)
                for bi in [4, 5, 0, 1, 2, 3]:
                    pb = ps8.tile([128, 512], F32, tag="A", bufs=3,
                                  name=f"pb_{t}_{bi}")
                    for ko in range(KC):
                        if bi < GQ:
                            w = wq_sb[:, ko, 128 * bi:128 * bi + 128]
                        elif bi == 4:
                            w = wkv_sb[:, ko, 0:HD]
                        else:
                            w = wkv_sb[:, ko, HD:2 * HD]
                        nc.tensor.matmul(pb[:, :], w, xcs[ko][:, :],
                                         start=(ko == 0), stop=(ko == KC - 1))
                    if bi == 5:
                        # v: evict with bias -> bf16, DMA-xbar to [tok, d]
                        vT_t = pr.tile([128, 512], BF16, tag="vT", bufs=2,
                                       name=f"vT_{t}")
                        nc.scalar.activation(vT_t[:, :], pb[:, :],
                                             AF.Identity,
                                             bias=bias_sb[:, 5:6])
                        for st4 in range(4):
                            nc.sync.dma_start(
                                v_sb[t][:, st4, :],
                                vT_t[:, 128 * st4:128 * st4 + 128],
                                transpose=True)
                    else:
                        bcol = bi if bi < GQ else 4
                        raw = pr.tile([128, 512], F32, tag="raw", bufs=3,
                                      name=f"raw_{t}_{bi}")
                        nc.scalar.activation(raw[:, :], pb[:, :],
                                             AF.Identity,
                                             bias=bias_sb[:, bcol:bcol + 1])
                        rot = pr.tile([128, 512], F32, tag="rot", bufs=2,
                                      name=f"rot_{t}_{bi}")
                        nc.vector.tensor_copy(rot[0:64, :], raw[64:128, :])
                        nc.vector.tensor_copy(rot[64:128, :], raw[0:64, :])
                        t1 = pr.tile([128, 512], F32, tag="t1", bufs=2,
                                     name=f"t1_{t}_{bi}")
                        nc.vector.tensor_mul(t1[:, :], rot[:, :],
                                             sin_sb[:, ts])
                        t2 = pr.tile([128, 512], F32, tag="t2", bufs=2,
                                     name=f"t2_{t}_{bi}")
                        nc.vector.tensor_mul(t2[:, :], raw[:, :],
                                             cos_sb[:, ts])
                        dst = qf_t[:, bi, :] if bi < GQ else kf[t][:, :]
                        nc.vector.tensor_add(dst, t1[:, :], t2[:, :])
                return qf_t

            # ============ interleaved projections + attention =============
            qf_tiles = [None] * TC
            for t in range(TC):
                qf_tiles[t] = proj_chunk(t)
                if t >= 1:
                    blocks = oproj_blocks(t - 2) if t >= 2 else []
                    attention(t - 1, qf_tiles[t - 1], blocks)
            attention(TC - 1, qf_tiles[TC - 1], oproj_blocks(TC - 2))
            # tail: o_proj of the last q-chunk, double-buffered across tags
            for i, blk in enumerate(oproj_blocks(TC - 1)):
                blk(tag=("OP" if i % 2 == 0 else "A"))


def _build():
    global _NC
    if _NC is None:
        nc = bacc.Bacc("TRN2", target_bir_lowering=False, debug=False,
                       num_devices=8)
        _emit(nc)
        nc.compile()
        _NC = nc
    return _NC


def _prep_inputs(x, wq, bq, wk, bk, wv, bv, wo, bo, cos, sin):
    """Host-side shard + layout prep. Core c = (g, b): g = c % 4, b = c // 4."""
    inv_sqrt_d = 1.0 / math.sqrt(HD)
    f32 = np.float32
    bf16 = ml_dtypes.bfloat16
    cosT = np.ascontiguousarray(cos.T.astype(f32))
    sinS = np.ascontiguousarray(sin.T.astype(f32))
    sinS[0:HD // 2] *= -1.0

    xTb = [np.ascontiguousarray(x[b].T.astype(bf16)) for b in range(B)]

    def pack(wT, nch):
        # [H or QD, M] -> [128, nch*M]: row p holds chunk-major slices
        m = wT.shape[1]
        return np.ascontiguousarray(
            wT.reshape(nch, 128, m).transpose(1, 0, 2).reshape(128, nch * m)
            .astype(bf16))

    in_maps = []
    for c in range(8):
        g, b = c % G, c // G
        wq_s = wq[QD * g:QD * (g + 1), :] * inv_sqrt_d
        bq_s = bq[QD * g:QD * (g + 1)] * inv_sqrt_d
        wk_s = wk[HD * g:HD * (g + 1), :]
        bk_s = bk[HD * g:HD * (g + 1)]
        wv_s = wv[HD * g:HD * (g + 1), :]
        bv_s = bv[HD * g:HD * (g + 1)]
        bias = np.zeros((128, 6), f32)
        bias[:, 0:4] = bq_s.reshape(GQ, HD).T
        bias[:, 4] = bk_s
        bias[:, 5] = bv_s
        wkvT = np.concatenate([wk_s.T, wv_s.T], axis=1)     # [H, 256]
        in_maps.append({
            "xT": xTb[b],
            "wqD": pack(wq_s.T, KC),
            "wkvD": pack(wkvT, KC),
            "woD": pack(wo[:, QD * g:QD * (g + 1)].T, GQ),
            "cosT": cosT,
            "sinS": sinS,
            "bqkv": bias,
            "onesd": np.ones((128, 128), bf16),
        })
    return in_maps


def run(inputs, trace=False):
    """Returns (full_output, BassKernelResults)."""
    inputs = {k: np.asarray(v) for k, v in inputs.items()}
    nc = _build()
    in_maps = _prep_inputs(**inputs)
    res = run_bass_kernel_spmd(nc, in_maps, core_ids=list(range(8)),
                               trace=trace)
    bo = inputs["bo"].astype(np.float64)
    out = np.empty((B, S, H), np.float32)
    for b in range(B):
        acc = np.zeros((S, H), np.float64)
        for g in range(G):
            acc += res.results[G * b + g]["outp"].astype(np.float64)
        out[b] = (acc + bo).astype(np.float32)
    return out, res


def kernel(**inputs):
    return run(inputs, trace=False)[0]
